# revision 1
# baseline (speedup 1.0000x reference)
"""MultiHeadAttention (pre-LN, residual) Trainium2 Bass kernel, 8 NeuronCores.

Problem: q,k,v [2, 2048, 1024], 16 heads x 64 dim, LN(q) -> QKV proj ->
softmax attention -> out proj -> +residual(q).

Sharding: core c owns tokens [512c, 512c+512) of the flattened [4096, 1024]
token axis (batch 0 = cores 0-3, batch 1 = cores 4-7).  All projections are
token-sharded (each core projects its 512 tokens for ALL heads).  The K / V
projections are then AllGathered *within each batch group of 4 cores*, so
every core holds its batch's full K^T / V and computes attention + output
projection for its own 512 query tokens.  No cross-core reduction is needed;
each core returns its 512 output rows.

Layout convention on device: "T layout" = features on partitions, tokens on
free axis.  PE matmuls contract over partitions, so:
  S^T tile [keys, q] = matmul(lhsT=K^T [dk, keys], rhs=Q^T [dk, q])
  O^T [dv, q]       += matmul(lhsT=V  [keys, dv],  rhs=exp(S^T) [keys, q])
  denom [1, q]      += matmul(lhsT=ones [keys, 1], rhs=exp(S^T) [keys, q])
Softmax is unnormalized exp (no max subtraction: S/tau is ~N(0,1), well
within fp32 exp range), normalized at the end by 1/denom broadcast via a
K=1 ones matmul.
"""

import numpy as np

N_CORES = 8
B, L, D = 2, 2048, 1024
H, DK, DV = 16, 64, 64
NT = B * L            # 4096 flattened tokens
TPC = NT // N_CORES   # 512 tokens per core
GROUP = 4             # cores per batch group
LB = L                # keys per batch (2048)
P = 128
NDT = D // P          # 8 d-tiles of 128
NMT = D // P          # 8 output-feature tiles
NTT = TPC // P        # 4 token tiles of 128 per core
NKT = LB // P         # 16 key tiles of 128 per batch
NHP = H // 2          # 8 head pairs
EPS = 1e-6
TAU_INV = 1.0 / float(np.sqrt(DK))

_CACHE = {}


def _np_reference(q, k, v, mask, w_q, w_k, w_v, w_o, ln_g, ln_b):
    """Pure-numpy fallback (only used if mask isn't all-ones)."""
    q64 = q.astype(np.float64)
    mu = q64.mean(-1, keepdims=True)
    var = q64.var(-1, keepdims=True)
    qn = (q64 - mu) / np.sqrt(var + EPS) * ln_g + ln_b
    Q = (qn @ w_q.T.astype(np.float64)).reshape(B, L, H, DK).transpose(0, 2, 1, 3)
    K = (k.astype(np.float64) @ w_k.T.astype(np.float64)).reshape(B, L, H, DK).transpose(0, 2, 1, 3)
    V = (v.astype(np.float64) @ w_v.T.astype(np.float64)).reshape(B, L, H, DV).transpose(0, 2, 1, 3)
    S = np.einsum("bhqd,bhkd->bhqk", Q / np.sqrt(DK), K)
    S = np.where(mask[None, None] == 0, -1e9, S)
    S = S - S.max(-1, keepdims=True)
    Pm = np.exp(S)
    Pm = Pm / Pm.sum(-1, keepdims=True)
    O = np.einsum("bhqk,bhkd->bhqd", Pm, V)
    O = O.transpose(0, 2, 1, 3).reshape(B, L, H * DV)
    out = O @ w_o.T.astype(np.float64) + q64
    return out.astype(np.float32)


def build_nc():
    import concourse.bass as bass
    import concourse.mybir as mybir
    import concourse.tile as tile
    from concourse import bacc
    from concourse.masks import make_identity

    f32 = mybir.dt.float32
    bf16 = mybir.dt.bfloat16

    nc = bacc.Bacc(num_devices=N_CORES)

    q_c = nc.declare_dram_parameter("q_c", [TPC, D], f32, isOutput=False)
    kT_c = nc.declare_dram_parameter("kT_c", [D, TPC], bf16, isOutput=False)
    vT_c = nc.declare_dram_parameter("vT_c", [D, TPC], bf16, isOutput=False)
    wgqT = nc.declare_dram_parameter("wgqT", [D, D], bf16, isOutput=False)
    wkT = nc.declare_dram_parameter("wkT", [D, D], bf16, isOutput=False)
    wvT = nc.declare_dram_parameter("wvT", [D, D], bf16, isOutput=False)
    woT = nc.declare_dram_parameter("woT", [D, D], bf16, isOutput=False)
    cq = nc.declare_dram_parameter("cq", [D], f32, isOutput=False)
    out_c = nc.declare_dram_parameter("out_c", [TPC, D], f32, isOutput=True)

    RG = [[0, 1, 2, 3], [4, 5, 6, 7]]

    with tile.TileContext(nc) as tc:
        with tc.tile_pool(name="dram", bufs=1, space="DRAM") as dram:
            kag_in = dram.tile([D, TPC], bf16)              # K^T shard (all heads, my tokens)
            vag_in = dram.tile([TPC, D], bf16)              # V natural shard
            kag_out = dram.tile([GROUP, D, TPC], bf16)
            vag_out = dram.tile([LB, D], bf16)

            with tc.tile_pool(name="singles", bufs=1) as singles:
                ident = singles.tile([P, P], f32)
                make_identity(nc, ident)
                ones_sb = singles.tile([P, P], bf16)
                nc.vector.memset(ones_sb, 1.0)
                ones_f32 = singles.tile([P, DK], f32)
                nc.vector.memset(ones_f32, 1.0)
                eps_sb = singles.tile([P, 1], f32)
                nc.vector.memset(eps_sb, EPS)
                cq_sb = singles.tile([P, NMT], f32)
                nc.sync.dma_start(out=cq_sb, in_=cq.rearrange("(mt p) -> p mt", p=P))

                # ---- persistent sbuf (live across phases) ----
                with tc.tile_pool(name="persist", bufs=1) as persist:
                    q_sb = persist.tile([P, NTT, D], f32)      # residual + LN input
                    qT_sb = persist.tile([P, NMT, TPC], bf16)   # Q^T (all heads, my tokens)
                    aO_sb = persist.tile([P, NHP, TPC], bf16)   # attn out^T (dv-concat, my tokens)

                    nc.sync.dma_start(
                        out=q_sb, in_=q_c.rearrange("(tt p) d -> p tt d", p=P)
                    )

                    # ================= Phase 1: K / V projections + AllGather ====
                    with tc.tile_pool(name="p1", bufs=1) as p1, \
                         tc.tile_pool(name="p1psum", bufs=3, space="PSUM") as p1psum:
                        wk_sb = p1.tile([P, NDT, D], bf16)
                        ktc_sb = p1.tile([P, NDT, TPC], bf16)
                        wkr = wkT.rearrange("(dt p) m -> p dt m", p=P)
                        ktr = kT_c.rearrange("(dt p) t -> p dt t", p=P)
                        for dt in range(NDT):
                            nc.sync.dma_start(out=wk_sb[:, dt, :], in_=wkr[:, dt, :])
                            nc.sync.dma_start(out=ktc_sb[:, dt, :], in_=ktr[:, dt, :])
                        kc_sb = p1.tile([P, NMT, TPC], bf16)
                        for mt in range(NMT):
                            ps = p1psum.tile([P, TPC], f32, tag="ps")
                            for dt in range(NDT):
                                nc.tensor.matmul(
                                    ps,
                                    wk_sb[:, dt, mt * P:(mt + 1) * P],
                                    ktc_sb[:, dt, :],
                                    start=(dt == 0),
                                    stop=(dt == NDT - 1),
                                )
                            nc.vector.tensor_copy(kc_sb[:, mt, :], ps)
                        nc.sync.dma_start(
                            out=kag_in.rearrange("(mt p) t -> p mt t", p=P),
                            in_=kc_sb,
                        )
                        nc.gpsimd.collective_compute(
                            "AllGather",
                            mybir.AluOpType.bypass,
                            replica_groups=RG,
                            ins=[kag_in[:, :].opt()],
                            outs=[kag_out[:, :, :].opt()],
                        )

                    # ================= Phase 1b: V projection + AllGather =========
                    with tc.tile_pool(name="p1v", bufs=1) as p1v, \
                         tc.tile_pool(name="p1vpsum", bufs=3, space="PSUM") as p1vpsum:
                        wv_sb = p1v.tile([P, NDT, D], bf16)
                        vtc_sb = p1v.tile([P, NDT, TPC], bf16)
                        wvr = wvT.rearrange("(dt p) m -> p dt m", p=P)
                        vtr = vT_c.rearrange("(dt p) t -> p dt t", p=P)
                        for dt in range(NDT):
                            nc.sync.dma_start(out=wv_sb[:, dt, :], in_=wvr[:, dt, :])
                            nc.sync.dma_start(out=vtc_sb[:, dt, :], in_=vtr[:, dt, :])
                        vn_sb = p1v.tile([P, NTT, D], bf16)
                        for tt in range(NTT):
                            for mc in range(2):  # dv-concat in two 512 chunks
                                ps = p1vpsum.tile([P, TPC], f32, tag="ps")
                                for dt in range(NDT):
                                    nc.tensor.matmul(
                                        ps,
                                        vtc_sb[:, dt, tt * P:(tt + 1) * P],
                                        wv_sb[:, dt, mc * 512:(mc + 1) * 512],
                                        start=(dt == 0),
                                        stop=(dt == NDT - 1),
                                    )
                                nc.vector.tensor_copy(
                                    vn_sb[:, tt, mc * 512:(mc + 1) * 512], ps
                                )
                        nc.sync.dma_start(
                            out=vag_in.rearrange("(tt p) d -> p tt d", p=P),
                            in_=vn_sb,
                        )
                        nc.gpsimd.collective_compute(
                            "AllGather",
                            mybir.AluOpType.bypass,
                            replica_groups=RG,
                            ins=[vag_in[:, :].opt()],
                            outs=[vag_out[:, :].opt()],
                        )


                    # ================= Phase 2: LayerNorm + Q projection ==========
                    with tc.tile_pool(name="p2", bufs=1) as p2, \
                         tc.tile_pool(name="p2w", bufs=1) as p2w, \
                         tc.tile_pool(name="p2s", bufs=4) as p2s, \
                         tc.tile_pool(name="p2psum", bufs=3, space="PSUM") as p2psum, \
                         tc.tile_pool(name="tpsum", bufs=2, space="PSUM") as tpsum:
                        qn_sb = p2.tile([P, NTT, D], f32)
                        for tt in range(NTT):
                            stats = p2s.tile([P, 2, 6], f32)
                            for sg in range(2):
                                nc.vector.bn_stats(
                                    out=stats[:, sg, :],
                                    in_=q_sb[:, tt, sg * 512:(sg + 1) * 512],
                                )
                            mv = p2s.tile([P, 2], f32)
                            nc.vector.bn_aggr(out=mv, in_=stats)
                            rstd = p2s.tile([P, 1], f32)
                            nc.scalar.activation(
                                out=rstd,
                                in_=mv[:, 1:2],
                                func=mybir.ActivationFunctionType.Sqrt,
                                bias=eps_sb,
                                scale=1.0,
                            )
                            nc.vector.reciprocal(out=rstd, in_=rstd)
                            nc.vector.tensor_scalar(
                                out=qn_sb[:, tt, :],
                                in0=q_sb[:, tt, :],
                                scalar1=mv[:, 0:1],
                                scalar2=rstd,
                                op0=mybir.AluOpType.subtract,
                                op1=mybir.AluOpType.mult,
                            )

                        # transpose qn -> qn^T [d on partitions, tokens free]
                        qnT_sb = p2.tile([P, NDT, TPC], bf16)
                        for tt in range(NTT):
                            for dt in range(NDT):
                                tp = tpsum.tile([P, P], f32, tag="tp")
                                nc.tensor.transpose(
                                    tp, qn_sb[:, tt, dt * P:(dt + 1) * P], ident
                                )
                                nc.vector.tensor_copy(
                                    qnT_sb[:, dt, tt * P:(tt + 1) * P], tp
                                )

                        wq_sb = p2w.tile([P, NDT, D], bf16)
                        nc.sync.dma_start(
                            out=wq_sb, in_=wgqT.rearrange("(dt p) m -> p dt m", p=P)
                        )
                        for mt in range(NMT):
                            ps = p2psum.tile([P, TPC], f32, tag="qps")
                            for dt in range(NDT):
                                nc.tensor.matmul(
                                    ps,
                                    wq_sb[:, dt, mt * P:(mt + 1) * P],
                                    qnT_sb[:, dt, :],
                                    start=(dt == 0),
                                    stop=(dt == NDT - 1),
                                )
                            # PSUM->SBUF + per-row bias (w_q @ ln_b)
                            nc.scalar.activation(
                                out=qT_sb[:, mt, :],
                                in_=ps,
                                func=mybir.ActivationFunctionType.Identity,
                                bias=cq_sb[:, mt:mt + 1],
                                scale=1.0,
                            )

                    # ================= Phase 3: attention =========================
                    with tc.tile_pool(name="kv", bufs=1) as kvp, \
                         tc.tile_pool(name="es", bufs=1) as es, \
                         tc.tile_pool(name="rp", bufs=3) as rp, \
                         tc.tile_pool(name="spsum", bufs=3, space="PSUM") as spsum, \
                         tc.tile_pool(name="opsum", bufs=1, space="PSUM") as opsum:
                        # Zero-padded full-array stationary operands and a
                        # 2-deep software pipeline over head pairs: S+exp for
                        # hp run 2 iterations ahead of the O matmuls (exp
                        # tiles buffered in SBUF), so ScalarE fills the
                        # AllGather-V wait and stays saturated after.
                        ksb_bufs = []
                        vsb_bufs = []
                        est_bufs = []
                        for i in range(2):
                            kb = kvp.tile([P, NKT, 2, P], bf16, name=f"ksb{i}")
                            nc.vector.memset(kb[DK:P, :, 0, :], 0.0)
                            nc.vector.memset(kb[0:DK, :, 1, :], 0.0)
                            vb = kvp.tile([P, NKT, 2, P], bf16, name=f"vsb{i}")
                            for h in range(2):
                                nc.vector.memset(vb[:, :, h, DK:DK + 1], 1.0)
                                nc.vector.memset(vb[:, :, h, DK + 1:P], 0.0)
                            ksb_bufs.append(kb)
                            vsb_bufs.append(vb)
                        for i in range(3):
                            eb = es.tile([P, NKT, 2, TPC], bf16, name=f"est{i}")
                            est_bufs.append(eb)

                        def emit_k_loads(hp):
                            ksb = ksb_bufs[hp % 2]
                            for h in range(2):
                                for r in range(GROUP):
                                    nc.sync.dma_start(
                                        out=ksb[
                                            h * DK:(h + 1) * DK,
                                            r * NTT:(r + 1) * NTT, h, :,
                                        ],
                                        in_=kag_out[
                                            r, hp * P + h * DK:hp * P + (h + 1) * DK, :
                                        ].rearrange("p (tc c) -> p tc c", c=P),
                                    )

                        def emit_v_loads(hp):
                            vsb = vsb_bufs[hp % 2]
                            for h in range(2):
                                nc.sync.dma_start(
                                    out=vsb[:, :, h, 0:DK],
                                    in_=vag_out[
                                        :, hp * P + h * DK:hp * P + (h + 1) * DK
                                    ].rearrange("(t p) c -> p t c", p=P),
                                )

                        def emit_s_exp(hp):
                            ksb = ksb_bufs[hp % 2]
                            est = est_bufs[hp % 3]
                            for ktp in range(NKT // 2):
                                sAB = [
                                    spsum.tile([P, 2, TPC], f32, tag="s", name=f"sA_{hp}_{ktp}"),
                                    spsum.tile([P, 2, TPC], f32, tag="s", name=f"sB_{hp}_{ktp}"),
                                ]
                                for half in range(2):
                                    kt = 2 * ktp + half
                                    for h in range(2):
                                        nc.tensor.matmul(
                                            sAB[h][:, half, :],
                                            ksb[:, kt, h, :],
                                            qT_sb[:, hp, :],
                                            start=True,
                                            stop=True,
                                        )
                                for h in range(2):
                                    nc.scalar.activation(
                                        out=est[:, 2 * ktp:2 * ktp + 2, h, :],
                                        in_=sAB[h],
                                        func=mybir.ActivationFunctionType.Exp,
                                        scale=TAU_INV,
                                    )

                        def emit_o(hp):
                            vsb = vsb_bufs[hp % 2]
                            est = est_bufs[hp % 3]
                            oAB = [
                                opsum.tile([P, TPC], f32, tag="oA", name=f"oA_{hp}"),
                                opsum.tile([P, TPC], f32, tag="oB", name=f"oB_{hp}"),
                            ]
                            for kt in range(NKT):
                                for h in range(2):
                                    nc.tensor.matmul(
                                        oAB[h],
                                        vsb[:, kt, h, :],
                                        est[:, kt, h, :],
                                        start=(kt == 0),
                                        stop=(kt == NKT - 1),
                                    )
                            return oAB

                        def emit_norm(hp, oAB):
                            rsb = rp.tile([P, 2, TPC], f32, tag="r", name=f"rsb{hp}")
                            for h in range(2):
                                nc.vector.reciprocal(
                                    out=rsb[0:1, h, :], in_=oAB[h][DK:DK + 1, :]
                                )
                            rbc = spsum.tile([P, TPC], f32, tag="s", name=f"rbc{hp}")
                            for h in range(2):
                                nc.tensor.matmul(
                                    rbc[DK * h:DK * (h + 1), :],
                                    ones_f32[0:1, :],
                                    rsb[0:1, h, :],
                                    start=True,
                                    stop=True,
                                    tile_position=(0, DK * h),
                                )
                            rbc_sb = rp.tile([P, TPC], f32, tag="rb", name=f"rbc_sb{hp}")
                            nc.vector.tensor_copy(rbc_sb, rbc)
                            for h in range(2):
                                nc.vector.tensor_mul(
                                    aO_sb[DK * h:DK * (h + 1), hp, :],
                                    oAB[h][0:DK, :],
                                    rbc_sb[DK * h:DK * (h + 1), :],
                                )

                        emit_k_loads(0)
                        emit_v_loads(0)
                        emit_s_exp(0)
                        emit_k_loads(1)
                        emit_v_loads(1)
                        emit_s_exp(1)
                        for hp in range(NHP):
                            if hp + 2 < NHP:
                                emit_k_loads(hp + 2)
                                emit_s_exp(hp + 2)
                            oAB = emit_o(hp)
                            if hp + 2 < NHP:
                                emit_v_loads(hp + 2)
                            emit_norm(hp, oAB)

                    # ================= Phase 4: out projection + residual =========
                    with tc.tile_pool(name="p4", bufs=1) as p4, \
                         tc.tile_pool(name="p4o", bufs=2) as p4o, \
                         tc.tile_pool(name="p4psum", bufs=2, space="PSUM") as p4psum:
                        wo_sb = p4.tile([P, NDT, D], bf16)
                        nc.sync.dma_start(
                            out=wo_sb, in_=woT.rearrange("(dt p) m -> p dt m", p=P)
                        )
                        for tt in range(NTT):
                            ob = p4o.tile([P, D], f32, tag="ob")
                            for mc in range(2):
                                ps = p4psum.tile([P, TPC], f32, tag="ops")
                                for dt in range(NDT):
                                    nc.tensor.matmul(
                                        ps,
                                        aO_sb[:, dt, tt * P:(tt + 1) * P],
                                        wo_sb[:, dt, mc * 512:(mc + 1) * 512],
                                        start=(dt == 0),
                                        stop=(dt == NDT - 1),
                                    )
                                nc.vector.tensor_add(
                                    ob[:, mc * 512:(mc + 1) * 512],
                                    ps,
                                    q_sb[:, tt, mc * 512:(mc + 1) * 512],
                                )
                            nc.sync.dma_start(
                                out=out_c[tt * P:(tt + 1) * P, :], in_=ob
                            )

    nc.compile()
    return nc


def _get_nc():
    if "nc" not in _CACHE:
        _CACHE["nc"] = build_nc()
    return _CACHE["nc"]


def make_in_maps(q, k, v, w_q, w_k, w_v, w_o, ln_g, ln_b):
    import ml_dtypes

    bf = ml_dtypes.bfloat16
    q2 = np.ascontiguousarray(q.reshape(NT, D), dtype=np.float32)
    kT = np.ascontiguousarray(k.reshape(NT, D).T.astype(bf))
    vT = np.ascontiguousarray(v.reshape(NT, D).T.astype(bf))
    wgqT = np.ascontiguousarray((w_q * ln_g[None, :]).T.astype(bf))
    wkT = np.ascontiguousarray(w_k.T.astype(bf))
    wvT = np.ascontiguousarray(w_v.T.astype(bf))
    woT = np.ascontiguousarray(w_o.T.astype(bf))
    cq = np.ascontiguousarray(w_q @ ln_b, dtype=np.float32)
    in_maps = []
    for c in range(N_CORES):
        sl = slice(c * TPC, (c + 1) * TPC)
        in_maps.append(
            {
                "q_c": q2[sl],
                "kT_c": np.ascontiguousarray(kT[:, sl]),
                "vT_c": np.ascontiguousarray(vT[:, sl]),
                "wgqT": wgqT,
                "wkT": wkT,
                "wvT": wvT,
                "woT": woT,
                "cq": cq,
            }
        )
    return in_maps


def run(inputs, trace=False, tmpdir=None):
    """Run the device kernel.  Returns (out [B, L, D], BassKernelResults)."""
    from concourse.bass_utils import run_bass_kernel_spmd

    nc = _get_nc()
    in_maps = make_in_maps(
        inputs["q"], inputs["k"], inputs["v"], inputs["w_q"], inputs["w_k"],
        inputs["w_v"], inputs["w_o"], inputs["ln_g"], inputs["ln_b"],
    )
    res = run_bass_kernel_spmd(
        nc, in_maps, list(range(N_CORES)), trace=trace, tmpdir=tmpdir
    )
    rows = np.concatenate([res.results[c]["out_c"] for c in range(N_CORES)], axis=0)
    return rows.reshape(B, L, D), res


def kernel(q, k, v, mask, w_q, w_k, w_v, w_o, ln_g, ln_b):
    q = np.asarray(q, dtype=np.float32)
    k = np.asarray(k, dtype=np.float32)
    v = np.asarray(v, dtype=np.float32)
    mask = np.asarray(mask)
    w_q = np.asarray(w_q, dtype=np.float32)
    w_k = np.asarray(w_k, dtype=np.float32)
    w_v = np.asarray(w_v, dtype=np.float32)
    w_o = np.asarray(w_o, dtype=np.float32)
    ln_g = np.asarray(ln_g, dtype=np.float32)
    ln_b = np.asarray(ln_b, dtype=np.float32)
    if not np.all(mask == 1):
        return _np_reference(q, k, v, mask, w_q, w_k, w_v, w_o, ln_g, ln_b)
    out, _ = run(
        {"q": q, "k": k, "v": v, "w_q": w_q, "w_k": w_k, "w_v": w_v,
         "w_o": w_o, "ln_g": ln_g, "ln_b": ln_b},
        trace=False,
    )
    return out



# revision 28
# speedup vs baseline: 1.1967x; 1.1967x over previous
"""MultiHeadAttention (pre-LN, residual) Trainium2 Bass kernel, 8 NeuronCores.

Problem: q,k,v [2, 2048, 1024], 16 heads x 64 dim, LN(q) -> QKV proj ->
softmax attention -> out proj -> +residual(q).

Sharding: core c owns tokens [512c, 512c+512) of the flattened [4096, 1024]
token axis (batch 0 = cores 0-3, batch 1 = cores 4-7).  Projections are
token-sharded; K^T / V are AllGathered (fp8, chunked) within each batch group
of 4 cores; each core runs attention + out projection for its 512 query
tokens over all 16 heads.

Matmuls are plain fp8 (1 output row / cycle; FWL weight loads).  S^T
contracts dk=64: the two heads of a pair run CONCURRENTLY in distinct PE
row-groups (tile_position (0,0) / (64,0)).  O^T contracts keys over 16
key-tiles with a 1/32-ones column (65th) in V producing the softmax
denominator in psum row 64.  exp() is split across engines by q columns:
ACT does true Exp into fp8e5m2 for q[0:QA]; DVE computes a Schraudolph-style
exp for q[QA:512] with one tensor_scalar (i8 = S*(4/ln2)/8 + 60) written
through an int8 bitcast of the e5m2 tile.  Normalization: approx-reciprocal
of psum row 64 + bf16 mask-matmul partition broadcast + DVE multiply.
"""

import numpy as np

N_CORES = 8
B, L, D = 2, 2048, 1024
H, DK, DV = 16, 64, 64
NT = B * L            # 4096 flattened tokens
TPC = NT // N_CORES   # 512 tokens per core
GROUP = 4             # cores per batch group
LB = L                # keys per batch (2048)
P = 128
NDT = D // P          # 8 d-tiles of 128
NTT = TPC // P        # 4 token tiles of 128 per core
NKT = LB // P         # 16 key tiles of 128 per batch
NKP = NKT // 2        # 8 key tile pairs
NHP = H // 2          # 8 head pairs
EPS = 1e-6

QA = 288              # q columns with true exp on ACT; rest Schraudolph on DVE
WS = 16.0             # host weight prescale into fp8 range
SCH_MUL = (4.0 / float(np.log(2.0))) / 8.0   # 0.72134752
SCH_ADD = 60.0        # e5m2 exponent bias 15 * 4

_CACHE = {}


def _np_reference(q, k, v, mask, w_q, w_k, w_v, w_o, ln_g, ln_b):
    """Pure-numpy fallback (only used if mask isn't all-ones)."""
    q64 = q.astype(np.float64)
    mu = q64.mean(-1, keepdims=True)
    var = q64.var(-1, keepdims=True)
    qn = (q64 - mu) / np.sqrt(var + EPS) * ln_g + ln_b
    Q = (qn @ w_q.T.astype(np.float64)).reshape(B, L, H, DK).transpose(0, 2, 1, 3)
    K = (k.astype(np.float64) @ w_k.T.astype(np.float64)).reshape(B, L, H, DK).transpose(0, 2, 1, 3)
    V = (v.astype(np.float64) @ w_v.T.astype(np.float64)).reshape(B, L, H, DV).transpose(0, 2, 1, 3)
    S = np.einsum("bhqd,bhkd->bhqk", Q / np.sqrt(DK), K)
    S = np.where(mask[None, None] == 0, -1e9, S)
    S = S - S.max(-1, keepdims=True)
    Pm = np.exp(S)
    Pm = Pm / Pm.sum(-1, keepdims=True)
    O = np.einsum("bhqk,bhkd->bhqd", Pm, V)
    O = O.transpose(0, 2, 1, 3).reshape(B, L, H * DV)
    out = O @ w_o.T.astype(np.float64) + q64
    return out.astype(np.float32)


def build_nc():
    import concourse.bass as bass
    import concourse.mybir as mybir
    import concourse.tile as tile
    from concourse import bacc
    from concourse.masks import make_identity

    f32 = mybir.dt.float32
    bf16 = mybir.dt.bfloat16
    f8e4 = mybir.dt.float8e4
    f8e5 = mybir.dt.float8e5
    i8 = mybir.dt.int8
    EXP = mybir.ActivationFunctionType.Exp

    nc = bacc.Bacc(num_devices=N_CORES)

    q_c = nc.declare_dram_parameter("q_c", [TPC, D], f32, isOutput=False)
    kT_c = nc.declare_dram_parameter("kT_c", [D, TPC], f8e4, isOutput=False)
    vT_c = nc.declare_dram_parameter("vT_c", [D, TPC], f8e4, isOutput=False)
    wq8 = nc.declare_dram_parameter("wq8", [D, D], f8e4, isOutput=False)
    wk8 = nc.declare_dram_parameter("wk8", [D, D], f8e4, isOutput=False)
    wv8 = nc.declare_dram_parameter("wv8", [D, D], f8e4, isOutput=False)
    wo8 = nc.declare_dram_parameter("wo8", [D, D], f8e4, isOutput=False)
    cq = nc.declare_dram_parameter("cq", [D], f32, isOutput=False)
    out_c = nc.declare_dram_parameter("out_c", [TPC, D], f32, isOutput=True)
    import os
    dbg = os.environ.get("KERNEL_DEBUG") == "1"
    if dbg:
        qnT_d = nc.declare_dram_parameter("qnT_d", [P, NDT, TPC], f8e4, isOutput=True)
        qT_d = nc.declare_dram_parameter("qT_d", [P, NHP, TPC], f8e4, isOutput=True)
        kT_d = nc.declare_dram_parameter("kT_d", [P, NHP, LB], f8e4, isOutput=True)
        v2_d = nc.declare_dram_parameter("v2_d", [P, H, NKT, 80], f8e5, isOutput=True)
        est_d = nc.declare_dram_parameter("est_d", [P, 4, NKT, TPC], f8e5, isOutput=True)
        aO_d = nc.declare_dram_parameter("aO_d", [P, NDT, TPC], f8e4, isOutput=True)
        den_d = nc.declare_dram_parameter("den_d", [1, H, TPC], f32, isOutput=True)
        r_d = nc.declare_dram_parameter("r_d", [65, NHP, TPC], f32, isOutput=True)

    RG = [[0, 1, 2, 3], [4, 5, 6, 7]]

    with tile.TileContext(nc) as tc:
        with tc.tile_pool(name="dram", bufs=1, space="DRAM") as dram:
            kag_in = dram.tile([D, TPC], f8e4, name="kag_in")
            kag_out = dram.tile([GROUP, D, TPC], f8e4, name="kag_out")
            vag_in = dram.tile([TPC, D], f8e5, name="vag_in")
            vag_out = dram.tile([GROUP, TPC, D], f8e5, name="vag_out")

            with tc.tile_pool(name="singles", bufs=1) as singles:
                ident = singles.tile([P, P], bf16)
                make_identity(nc, ident)
                # partition-broadcast mask: row0 -> out parts 0:64, row64 -> 64:128
                bcm = singles.tile([65, P], bf16)
                nc.vector.memset(bcm[0:64, :], 0.0)
                nc.vector.memset(bcm[64:65, 0:DK], 0.0)
                nc.vector.memset(bcm[0:1, 0:DK], 1.0)
                nc.vector.memset(bcm[64:65, DK:P], 1.0)
                bco = singles.tile([1, P], bf16)
                nc.vector.memset(bco, 1.0)
                r2 = singles.tile([1, 2, TPC], f32)
                r2src = singles.tile([1, 2, TPC], f32)
                r2b = singles.tile([1, 2, TPC], bf16)
                cq_sb = singles.tile([P, NDT], f32)
                nc.sync.dma_start(out=cq_sb, in_=cq.rearrange("(t p) -> p t", p=P))
                eps_sb = singles.tile([P, 1], f32)
                nc.vector.memset(eps_sb, EPS)

                with tc.tile_pool(name="persist", bufs=1) as persist:
                    q_sb = persist.tile([P, NTT, D], f32)          # residual + LN in
                    qn_sb = persist.tile([P, NTT, D], bf16)        # LN out
                    qnT = persist.tile([P, NDT, TPC], f8e4)        # qn^T
                    qT = persist.tile([P, NHP, TPC], f8e4)         # Q^T by head pair
                    kT = persist.tile([P, NHP, LB], f8e4)          # K^T full batch
                    v2 = persist.tile([P, H, NKT, 80], f8e5)       # V + 1/32 ones col
                    aO = persist.tile([P, NDT, TPC], f8e4)         # attn out (x32/den)
                    kag_sb = persist.tile([P, NDT, TPC], f8e4)     # K^T pre-AG staging
                    vag_sb = persist.tile([P, NTT, D], f8e5)       # V pre-AG staging
                    wk_sb = persist.tile([P, NDT, D], f8e4)
                    wv_sb = persist.tile([P, NDT, D], f8e4)
                    wq_sb = persist.tile([P, NDT, D], f8e4)
                    wo_sb = persist.tile([P, NDT, D], f8e4)
                    kc_sb = persist.tile([P, NDT, TPC], f8e4)      # k^T shard
                    vc_sb = persist.tile([P, NDT, TPC], f8e4)      # v^T shard

                    # ones column of V (denominator): 1/32 so r = 32/den
                    nc.gpsimd.memset(v2[:, :, :, 64:65], 1.0 / 32.0)
                    nc.gpsimd.memset(v2[:, :, :, 65:66], 0.0)

                    # ---- input DMAs (K-proj operands first) ----
                    nc.sync.dma_start(out=wk_sb, in_=wk8.rearrange("(dt p) f -> p dt f", p=P))
                    nc.sync.dma_start(out=kc_sb, in_=kT_c.rearrange("(dt p) t -> p dt t", p=P))
                    nc.sync.dma_start(out=q_sb, in_=q_c.rearrange("(tt p) d -> p tt d", p=P))
                    nc.sync.dma_start(out=wq_sb, in_=wq8.rearrange("(dt p) f -> p dt f", p=P))
                    nc.sync.dma_start(out=wv_sb, in_=wv8.rearrange("(dt p) f -> p dt f", p=P))
                    nc.sync.dma_start(out=vc_sb, in_=vT_c.rearrange("(dt p) t -> p dt t", p=P))
                    nc.sync.dma_start(out=wo_sb, in_=wo8.rearrange("(dt p) m -> p dt m", p=P))

                    # =========== Phase 1: LN + K proj/AG + Q proj + V proj/AG ===
                    with tc.tile_pool(name="p1s", bufs=4) as p1s, \
                         tc.tile_pool(name="ppsum", bufs=3, space="PSUM") as ppsum, \
                         tc.tile_pool(name="tpsum", bufs=2, space="PSUM") as tpsum:

                        # LayerNorm on DVE (overlaps K proj on PE)
                        for tt in range(NTT):
                            stats = p1s.tile([P, 2, 6], f32, tag="st")
                            for sg in range(2):
                                nc.vector.bn_stats(
                                    out=stats[:, sg, :],
                                    in_=q_sb[:, tt, sg * 512:(sg + 1) * 512],
                                )
                            mv = p1s.tile([P, 2], f32, tag="mv")
                            nc.vector.bn_aggr(out=mv, in_=stats)
                            rstd = p1s.tile([P, 1], f32, tag="rs")
                            nc.scalar.activation(
                                out=rstd, in_=mv[:, 1:2],
                                func=mybir.ActivationFunctionType.Sqrt,
                                bias=eps_sb, scale=1.0,
                            )
                            nc.vector.reciprocal(out=rstd, in_=rstd)
                            nc.vector.tensor_scalar(
                                out=qn_sb[:, tt, :], in0=q_sb[:, tt, :],
                                scalar1=mv[:, 0:1], scalar2=rstd,
                                op0=mybir.AluOpType.subtract,
                                op1=mybir.AluOpType.mult,
                            )

                        # K projection: psum tile t = features [128t,128t+128)
                        for t in range(NDT):
                            ps = ppsum.tile([P, TPC], f32, tag="pp")
                            for dt in range(NDT):
                                nc.tensor.matmul(
                                    ps,
                                    wk_sb[:, dt, t * P:(t + 1) * P],
                                    kc_sb[:, dt, :],
                                    start=(dt == 0), stop=(dt == NDT - 1),
                                )
                            nc.scalar.mul(kag_sb[:, t, :], ps, 1.0 / WS)
                        nc.sync.dma_start(
                            out=kag_in.rearrange("(hp p) t -> p hp t", p=P),
                            in_=kag_sb,
                        )
                        nc.gpsimd.collective_compute(
                            "AllGather", mybir.AluOpType.bypass,
                            replica_groups=RG,
                            ins=[kag_in[:, :].opt()],
                            outs=[kag_out[:, :, :].opt()],
                        )
                        for r in range(GROUP):
                            nc.sync.dma_start(
                                out=kT[:, :, r * TPC:(r + 1) * TPC],
                                in_=kag_out[r].rearrange("(hp p) t -> p hp t", p=P),
                            )

                        # V projection: psum tile (fc, tt) = [128 tok, 512 feat]
                        for fc in range(2):
                            for tt in range(NTT):
                                ps = ppsum.tile([P, TPC], f32, tag="pp")
                                for dt in range(NDT):
                                    nc.tensor.matmul(
                                        ps,
                                        vc_sb[:, dt, tt * P:(tt + 1) * P],
                                        wv_sb[:, dt, fc * 512:(fc + 1) * 512],
                                        start=(dt == 0), stop=(dt == NDT - 1),
                                    )
                                nc.vector.tensor_scalar_mul(
                                    vag_sb[:, tt, fc * 512:(fc + 1) * 512], ps, 1.0 / WS
                                )
                        nc.sync.dma_start(
                            out=vag_in.rearrange("(tt p) f -> p tt f", p=P),
                            in_=vag_sb,
                        )
                        nc.gpsimd.collective_compute(
                            "AllGather", mybir.AluOpType.bypass,
                            replica_groups=RG,
                            ins=[vag_in[:, :].opt()],
                            outs=[vag_out[:, :, :].opt()],
                        )
                        # v2[p, h, kt, dv]; key = g*512 + tt*128 + p
                        nc.sync.dma_start(
                            out=v2[:, :, :, 0:DV],
                            in_=vag_out.rearrange(
                                "g (tt p) (h dv) -> p h (g tt) dv",
                                p=P, dv=DV,
                            ),
                        )

                        # qn^T: PE transpose (bf16) + ACT evict to fp8
                        for dt in range(NDT):
                            tp = tpsum.tile([P, TPC], bf16, tag="tp")
                            for tt in range(NTT):
                                nc.tensor.transpose(
                                    tp[:, tt * P:(tt + 1) * P],
                                    qn_sb[:, tt, dt * P:(dt + 1) * P],
                                    ident,
                                )
                            nc.scalar.mul(qnT[:, dt, :], tp, 1.0)

                        # Q projection + bias
                        for t in range(NDT):
                            ps = ppsum.tile([P, TPC], f32, tag="pp")
                            for dt in range(NDT):
                                nc.tensor.matmul(
                                    ps,
                                    wq_sb[:, dt, t * P:(t + 1) * P],
                                    qnT[:, dt, :],
                                    start=(dt == 0), stop=(dt == NDT - 1),
                                )
                            nc.scalar.activation(
                                out=qT[:, t, :], in_=ps,
                                func=mybir.ActivationFunctionType.Identity,
                                bias=cq_sb[:, t:t + 1], scale=1.0 / WS,
                            )

                    # =========== Phase 2: attention ============================
                    if dbg:
                        den_dbg = persist.tile([1, H, TPC], f32)
                        r_dbg = persist.tile([65, NHP, TPC], f32)
                    with tc.tile_pool(name="est", bufs=1) as estp, \
                         tc.tile_pool(name="rbp", bufs=2) as rbp, \
                         tc.tile_pool(name="spsum", bufs=1, space="PSUM") as spsum, \
                         tc.tile_pool(name="opsum", bufs=3, space="PSUM") as opsum, \
                         tc.tile_pool(name="rpsum", bufs=1, space="PSUM") as rpsum:

                        est_bufs = [estp.tile([P, NKT, TPC], f8e5, name=f"est{j}")
                                    for j in range(4)]
                        o_ps = {}

                        def emit_s_exp(hp):
                            # two heads of the pair in concurrent PE row-groups
                            ests = [est_bufs[(2 * hp) % 4], est_bufs[(2 * hp + 1) % 4]]
                            for tp2 in range(NKP):
                                sps = [
                                    spsum.tile([P, 2, TPC], f32, tag="sA",
                                               name=f"sA_{hp}_{tp2}"),
                                    spsum.tile([P, 2, TPC], f32, tag="sB",
                                               name=f"sB_{hp}_{tp2}"),
                                ]
                                for half in range(2):
                                    kt = 2 * tp2 + half
                                    for par in range(2):
                                        nc.tensor.matmul(
                                            sps[par][:, half, :],
                                            kT[64 * par:64 * par + 64, hp,
                                               kt * P:(kt + 1) * P],
                                            qT[64 * par:64 * par + 64, hp, :],
                                            start=True, stop=True,
                                            tile_position=(64 * par, 0),
                                        )
                                for par in range(2):
                                    if QA > 0:
                                        nc.scalar.activation(
                                            out=ests[par][:, 2 * tp2:2 * tp2 + 2, 0:QA],
                                            in_=sps[par][:, :, 0:QA],
                                            func=EXP, scale=0.125,
                                        )
                                    if QA < TPC:
                                        nc.vector.tensor_scalar(
                                            out=ests[par][:, 2 * tp2:2 * tp2 + 2, QA:TPC].bitcast(i8),
                                            in0=sps[par][:, :, QA:TPC],
                                            scalar1=SCH_MUL, scalar2=SCH_ADD,
                                            op0=mybir.AluOpType.mult,
                                            op1=mybir.AluOpType.add,
                                        )

                        def emit_o(h):
                            est = est_bufs[h % 4]
                            ops = opsum.tile([P, TPC], f32, tag="o", name=f"o_{h}")
                            for kt in range(NKT):
                                nc.tensor.matmul(
                                    ops[0:DV + 2, :],
                                    v2[:, h, kt, 0:DV + 2],
                                    est[:, kt, :],
                                    start=(kt == 0), stop=(kt == NKT - 1),
                                )
                            o_ps[h] = ops

                        def emit_norm(h):
                            # h odd: normalize heads h-1, h
                            opsA, opsB = o_ps.pop(h - 1), o_ps.pop(h)
                            if dbg:
                                nc.vector.tensor_copy(den_dbg[0:1, h - 1, :], opsA[DV:DV + 1, :])
                                nc.vector.tensor_copy(den_dbg[0:1, h, :], opsB[DV:DV + 1, :])
                            nc.vector.tensor_copy(r2src[0:1, 0, :], opsA[DV:DV + 1, :])
                            nc.vector.tensor_copy(r2src[0:1, 1, :], opsB[DV:DV + 1, :])
                            nc.vector.reciprocal_approx_fast(
                                out=r2[0:1, 0, :], in_=r2src[0:1, 0, :])
                            nc.vector.reciprocal_approx_fast(
                                out=r2[0:1, 1, :], in_=r2src[0:1, 1, :])
                            if dbg:
                                nc.vector.tensor_copy(r_dbg[0:1, h // 2, :], r2[0:1, 0, :])
                                nc.vector.tensor_copy(r_dbg[64:65, h // 2, :], r2[0:1, 1, :])
                            nc.vector.tensor_copy(r2b[:, :, :], r2[:, :, :])
                            d = h // 2
                            rbcA = rpsum.tile([P, TPC], f32, tag="rb", name=f"rbA_{h}")
                            nc.tensor.matmul(
                                rbcA[:, :], bco[:, :], r2b[0:1, 0, :],
                                start=True, stop=True,
                            )
                            rbsA = rbp.tile([P, TPC], bf16, tag="rs", name=f"rsA_{h}")
                            nc.scalar.mul(rbsA[:, :], rbcA[:, :], 1.0)
                            nc.vector.tensor_tensor(
                                out=aO[0:DV, d, :], in0=opsA[0:DV, :],
                                in1=rbsA[0:DV, :], op=mybir.AluOpType.mult,
                            )
                            rbcB = rpsum.tile([P, TPC], f32, tag="rb", name=f"rbB_{h}")
                            nc.tensor.matmul(
                                rbcB[:, :], bco[:, :], r2b[0:1, 1, :],
                                start=True, stop=True,
                            )
                            rbsB = rbp.tile([P, TPC], bf16, tag="rs", name=f"rsB_{h}")
                            nc.scalar.mul(rbsB[:, :], rbcB[:, :], 1.0)
                            nc.vector.tensor_tensor(
                                out=aO[DV:P, d, :], in0=opsB[0:DV, :],
                                in1=rbsB[DV:P, :], op=mybir.AluOpType.mult,
                            )

                        emit_s_exp(0)
                        for hp in range(NHP):
                            if hp + 1 < NHP:
                                emit_s_exp(hp + 1)
                            emit_o(2 * hp)
                            emit_o(2 * hp + 1)
                            emit_norm(2 * hp + 1)

                        if dbg:
                            nc.sync.dma_start(out=qnT_d[:, :, :], in_=qnT)
                            nc.sync.dma_start(out=qT_d[:, :, :], in_=qT)
                            nc.sync.dma_start(out=kT_d[:, :, :], in_=kT)
                            nc.sync.dma_start(out=v2_d[:, :, :, :], in_=v2)
                            for j in range(4):
                                nc.sync.dma_start(out=est_d[:, j, :, :], in_=est_bufs[j])
                            nc.sync.dma_start(out=aO_d[:, :, :], in_=aO)
                            nc.sync.dma_start(out=den_d[:, :, :], in_=den_dbg)
                            nc.sync.dma_start(out=r_d[:, :, :], in_=r_dbg)

                    # =========== Phase 3: out projection + residual ============
                    with tc.tile_pool(name="p4o", bufs=2) as p4o, \
                         tc.tile_pool(name="fpsum", bufs=2, space="PSUM") as fpsum:
                        for tt in range(NTT):
                            for mc in range(2):
                                fps = fpsum.tile([P, TPC], f32, tag="f")
                                for dt in range(NDT):
                                    nc.tensor.matmul(
                                        fps,
                                        aO[:, dt, tt * P:(tt + 1) * P],
                                        wo_sb[:, dt, mc * 512:(mc + 1) * 512],
                                        start=(dt == 0), stop=(dt == NDT - 1),
                                    )
                                ob = p4o.tile([P, TPC], f32, tag="ob")
                                nc.vector.scalar_tensor_tensor(
                                    out=ob, in0=fps, scalar=1.0 / (WS * 32.0),
                                    in1=q_sb[:, tt, mc * 512:(mc + 1) * 512],
                                    op0=mybir.AluOpType.mult,
                                    op1=mybir.AluOpType.add,
                                )
                                nc.sync.dma_start(
                                    out=out_c[tt * P:(tt + 1) * P, mc * 512:(mc + 1) * 512],
                                    in_=ob,
                                )

    nc.compile()
    return nc


def _get_nc():
    if "nc" not in _CACHE:
        _CACHE["nc"] = build_nc()
    return _CACHE["nc"]


def make_in_maps(q, k, v, w_q, w_k, w_v, w_o, ln_g, ln_b):
    import ml_dtypes

    e4 = ml_dtypes.float8_e4m3
    q2 = np.ascontiguousarray(q.reshape(NT, D), dtype=np.float32)
    kT8 = np.ascontiguousarray(k.reshape(NT, D).T.astype(e4))
    vT8 = np.ascontiguousarray(v.reshape(NT, D).T.astype(e4))
    wgq = w_q * ln_g[None, :]
    wq8 = np.ascontiguousarray((WS * wgq).T.astype(e4))
    wk8 = np.ascontiguousarray((WS * w_k).T.astype(e4))
    wv8 = np.ascontiguousarray((WS * w_v).T.astype(e4))
    wo8 = np.ascontiguousarray((WS * w_o).T.astype(e4))
    cq = np.ascontiguousarray(w_q @ ln_b, dtype=np.float32)
    in_maps = []
    for c in range(N_CORES):
        sl = slice(c * TPC, (c + 1) * TPC)
        in_maps.append(
            {
                "q_c": q2[sl],
                "kT_c": np.ascontiguousarray(kT8[:, sl]),
                "vT_c": np.ascontiguousarray(vT8[:, sl]),
                "wq8": wq8,
                "wk8": wk8,
                "wv8": wv8,
                "wo8": wo8,
                "cq": cq,
            }
        )
    return in_maps


def run(inputs, trace=False, tmpdir=None):
    """Run the device kernel.  Returns (out [B, L, D], BassKernelResults)."""
    from concourse.bass_utils import run_bass_kernel_spmd

    nc = _get_nc()
    in_maps = make_in_maps(
        inputs["q"], inputs["k"], inputs["v"], inputs["w_q"], inputs["w_k"],
        inputs["w_v"], inputs["w_o"], inputs["ln_g"], inputs["ln_b"],
    )
    res = run_bass_kernel_spmd(
        nc, in_maps, list(range(N_CORES)), trace=trace, tmpdir=tmpdir
    )
    rows = np.concatenate([res.results[c]["out_c"] for c in range(N_CORES)], axis=0)
    return rows.reshape(B, L, D), res


def kernel(q, k, v, mask, w_q, w_k, w_v, w_o, ln_g, ln_b):
    q = np.asarray(q, dtype=np.float32)
    k = np.asarray(k, dtype=np.float32)
    v = np.asarray(v, dtype=np.float32)
    mask = np.asarray(mask)
    w_q = np.asarray(w_q, dtype=np.float32)
    w_k = np.asarray(w_k, dtype=np.float32)
    w_v = np.asarray(w_v, dtype=np.float32)
    w_o = np.asarray(w_o, dtype=np.float32)
    ln_g = np.asarray(ln_g, dtype=np.float32)
    ln_b = np.asarray(ln_b, dtype=np.float32)
    if not np.all(mask == 1):
        return _np_reference(q, k, v, mask, w_q, w_k, w_v, w_o, ln_g, ln_b)
    out, _ = run(
        {"q": q, "k": k, "v": v, "w_q": w_q, "w_k": w_k, "w_v": w_v,
         "w_o": w_o, "ln_g": ln_g, "ln_b": ln_b},
        trace=False,
    )
    return out
